# revision 15
# baseline (speedup 1.0000x reference)
"""ActionEncoder Trainium2 kernel (8 NeuronCores, expert-parallel).

Strategy:
- Host groups the 32768 flat actions by action_type (1=pick, 2=transport,
  3=move; type 0 rows are wait_emb and never touch the device), splits each
  group evenly across the 8 cores, and pads each per-core shard to a
  multiple of 128 (pad rows use table row 0 and are discarded).
- Everything on-device is bf16 (rel-err budget 2e-2; measured ~4e-3).
- Startup: the GPSIMD ucode library reload gates SWDGE gathers for the
  first ~14us of the kernel, so the host pre-gathers each expert's FIRST
  chunk (512 rows) and packs it with that expert's W1|W2|biases into ONE
  wide-row bf16 param per expert ([128, ~5-9K] -> 10-18KB DGE descriptors,
  bandwidth-bound); the MLPs start on those while the GPSIMD farm preps
  dma_gather descriptors for the remaining chunks (<=8 gathers so the 8
  SWDGE completion semaphores never recycle). Dummy warmup matmuls on a
  memset tile pull the PE DVFS ladder (~6us per p-state step) early.
- Two bf16 GEMMs per expert with LeakyReLU(0.01)+bias fused on ScalarE
  (hidden stays bf16). Output is written feature-major [256, C] bf16 and
  un-transposed/scattered on the host; the smallest chunk drains last.
- Weights/tables are replicated per core. One SPMD NEFF for all 8 cores.
"""
import sys

import numpy as np

sys.path.insert(0, "/opt/trn_rl_repo")

import ml_dtypes

import concourse.bass as bass
import concourse.bacc as bacc
import concourse.mybir as mybir
import concourse.tile as tile
from concourse import library_config
from concourse.bass_utils import run_bass_kernel_spmd


def _ensure_axon_hooks():
    """Some images lack antenv.axon_hooks; register the ctypes NTFF hook
    shim so run_bass_kernel_spmd's trace path works instead of crashing."""
    try:
        import antenv.axon_hooks  # noqa: F401
        return
    except ImportError:
        pass
    import types

    try:
        import antenv
        from trn_agent_boot.trn_boot import _ntff_profile_via_ctypes

        hook = _ntff_profile_via_ctypes("/opt/axon/libaxon_pjrt.so")
    except Exception:
        return
    mod = types.ModuleType("antenv.axon_hooks")
    state = {"hook": hook}
    mod.get_axon_ntff_profile_hook = lambda: state["hook"]
    mod.set_axon_ntff_profile_hook = lambda h: state.update(hook=h)
    sys.modules["antenv.axon_hooks"] = mod
    antenv.axon_hooks = mod


_ensure_axon_hooks()

D = 256
HID = 512
OUT = 256
NTAB = 8192
NCORES = 8
NA = 512  # max actions per compute chunk (matmul moving dim)
FP32 = mybir.dt.float32
BF16 = mybir.dt.bfloat16
INT16 = mybir.dt.int16

LAST_RESULT = None  # BassKernelResults of the most recent kernel() call

# (name, gathered tables, layer-1 K); trans/move first so the first
# pre-gathered chunks are the cheap ones and pick's device gathers have
# the longest runway.
EXPERTS = (
    ("trans", ("agv", "mach"), 2 * D),
    ("move", ("agv", "mach"), 2 * D),
    ("pick", ("agv", "from", "to", "mach"), 4 * D),
)
TABLE_OF = {"agv": "emb_AGV", "from": "emb_operation", "to": "emb_operation", "mach": "emb_machine"}


def _dev_chunks(c):
    """Device-gathered chunks of (pos, n) covering [NA, c): <=NA each,
    multiples of 128. Chunk 0 ([0, min(NA, c))) is host-pre-gathered."""
    rem = c - min(NA, c)
    if rem == 0:
        return []
    nch = -(-rem // NA)
    base = rem // nch // 128 * 128
    sizes = [base] * (nch - 1) + [rem - base * (nch - 1)]
    sizes.sort(reverse=True)  # largest first -> smallest chunk drains last
    out = []
    pos = NA
    for n in sizes:
        out.append((pos, n))
        pos += n
    return out


def _sched(caps):
    """Interleaved (expert, pos, n, is_dev) chunk order, round-robin across
    experts: every expert's chunk 0 first, then device chunks."""
    lists = {}
    for name, _, _ in EXPERTS:
        c = caps[name]
        lists[name] = [(0, min(NA, c), False)] + [(p, n, True) for p, n in _dev_chunks(c)]
    order = []
    while any(lists.values()):
        for name, _, _ in EXPERTS:
            if lists[name]:
                pos, n, dev = lists[name].pop(0)
                order.append((name, pos, n, dev))
    return order


def _build(caps):
    """Emit the per-core BIR. caps = dict expert -> padded capacity."""
    nc = bacc.Bacc(num_swdge_queues=4)

    tabs = {}
    for tn in ("emb_operation", "emb_machine", "emb_AGV"):
        tabs[tn] = nc.declare_dram_parameter(f"{tn}_b", [NTAB, D], BF16, isOutput=False)

    # one packed bf16 param per expert, wide partition rows so the hw DGE
    # is bandwidth-bound, not descriptor-rate-bound:
    #   [x0 (K/128*base) | W1 (K/128*HID) | W2 (HID/128*OUT) | biases (12)]
    # biases are fp32 bitcast into 12 bf16 columns (b1: HID/128, b2: OUT/128)
    params = {}
    nb = HID // 128 + OUT // 128
    pk_off = {}
    for name, tables, K in EXPERTS:
        c = caps[name]
        base = min(NA, c)
        x0sz = (K // 128) * base
        w1sz = (K // 128) * HID
        w2sz = (HID // 128) * OUT
        pk_off[name] = (x0sz, x0sz + w1sz, x0sz + w1sz + w2sz)
        params[f"{name}_pk"] = nc.declare_dram_parameter(
            f"{name}_pk", [128, x0sz + w1sz + w2sz + 2 * nb], BF16, isOutput=False
        )
        params[f"{name}_outT"] = nc.declare_dram_parameter(f"{name}_outT", [OUT, c], BF16, isOutput=True)

    sched = _sched(caps)
    dev_sched = [s for s in sched if s[3]]

    # wrapped int16 index segments, only for device-gathered rows [NA, c)
    seg_off = {}
    off = 0
    for name, tables, K in EXPERTS:
        dev_rows = caps[name] - min(NA, caps[name])
        for t in tables:
            seg_off[(name, t)] = off
            off += dev_rows // 16
    if off:
        params["idx_all"] = nc.declare_dram_parameter("idx_all", [128, off], INT16, isOutput=False)

    qrr = [0]  # SWDGE queue round-robin counter

    with tile.TileContext(nc) as tc:
        with (
            tc.tile_pool(name="wp", bufs=1) as wp,
            tc.tile_pool(name="xp", bufs=2) as xp,
            tc.tile_pool(name="ps", bufs=1, space="PSUM") as ps,
        ):
            if dev_sched:
                nc.gpsimd.load_library(library_config.mlp)

            # --- warm tile built on-chip (no DMA), packed expert params in
            # first-use order, idx after the first expert's param ---
            wpe = wp.tile([128, NA], BF16, name="warm_pe")
            nc.vector.memset(wpe[:], 1.0)
            PK = {}
            idx_all = None
            for i, (name, tables, K) in enumerate(EXPERTS):
                cols = pk_off[name][2] + 2 * nb
                PK[name] = wp.tile([128, cols], BF16, name=f"pk_{name}")
                nc.sync.dma_start(out=PK[name][:], in_=params[f"{name}_pk"][:])
                if i == 0 and dev_sched:
                    idx_all = wp.tile([128, off], INT16, name="idx_all")
                    nc.sync.dma_start(out=idx_all[:], in_=params["idx_all"][:])

            # --- device gather preps, issued up-front in compute order ---
            gh = {}
            for name, pos, n, _ in dev_sched:
                tables = dict((e[0], e[1]) for e in EXPERTS)[name]
                base = min(NA, caps[name])
                for t in tables:
                    g = wp.tile([128, D // 128, n], BF16, name=f"g_{name}_{t}_{pos}")
                    so = seg_off[(name, t)]
                    nc.gpsimd.dma_gather(
                        g[:],
                        tabs[TABLE_OF[t]][:],
                        idx_all[:, so + (pos - base) // 16 : so + (pos + n - base) // 16],
                        n,
                        n,
                        D,
                        transpose=True,
                        queue_num=qrr[0] % 4,
                    )
                    qrr[0] += 1
                    gh[(name, t, pos)] = g

            # --- PE p-state warmup: dummy matmuls ramp the clock to 2.4GHz
            # while the first expert's weights/activations stream in ---
            for wi in range(9):
                pw = ps.tile([128, NA], FP32, space="PSUM", tag="pw", bufs=1, name="pw")
                nc.tensor.matmul(
                    out=pw[:], lhsT=wpe[:, :128], rhs=wpe[:], start=True, stop=True
                )

            # --- compute, chunk by chunk ---
            last_chunk = sched[-1]
            expert_of = dict((e[0], e) for e in EXPERTS)
            eidx = dict((e[0], i) for i, e in enumerate(EXPERTS))
            for name, pos, n, dev in sched:
                _, tables, K = expert_of[name]
                base = min(NA, caps[name])
                o1, o2, ob = pk_off[name]

                def rhs1(k):
                    if dev:
                        return gh[(name, tables[k // 2], pos)][:, k % 2, :n]
                    return PK[name][:, k * base : k * base + n]

                def bias_ap(j):
                    return PK[name][:, ob + 2 * j : ob + 2 * j + 2].bitcast(FP32)

                # layer 1: H = Prelu(X @ W1 + b1), feature-major
                hT = xp.tile([128, HID // 128, NA], BF16, tag="hT", name=f"hT_{name}")
                for m in range(HID // 128):
                    p1 = ps.tile([128, NA], FP32, space="PSUM", tag="p1", bufs=3, name="p1")
                    for k in range(K // 128):
                        nc.tensor.matmul(
                            out=p1[:, :n],
                            lhsT=PK[name][:, o1 + k * HID + m * 128 : o1 + k * HID + (m + 1) * 128],
                            rhs=rhs1(k),
                            start=(k == 0),
                            stop=(k == K // 128 - 1),
                        )
                    nc.scalar.activation(
                        out=hT[:, m, :n],
                        in_=p1[:, :n],
                        func=mybir.ActivationFunctionType.Prelu,
                        bias=bias_ap(m),
                        scale=1.0,
                        alpha=0.01,
                    )

                # layer 2: O = H @ W2 + b2, feature-major
                osb = xp.tile([128, OUT // 128, NA], BF16, tag="o", name=f"o_{name}")
                for m2 in range(OUT // 128):
                    p2 = ps.tile([128, NA], FP32, space="PSUM", tag="p2", bufs=3, name="p2")
                    for k2 in range(HID // 128):
                        nc.tensor.matmul(
                            out=p2[:, :n],
                            lhsT=PK[name][:, o2 + k2 * OUT + m2 * 128 : o2 + k2 * OUT + (m2 + 1) * 128],
                            rhs=hT[:, k2, :n],
                            start=(k2 == 0),
                            stop=(k2 == HID // 128 - 1),
                        )
                    nc.vector.tensor_tensor(
                        out=osb[:, m2, :n],
                        in0=p2[:, :n],
                        in1=bias_ap(HID // 128 + m2).to_broadcast([128, n]),
                        op=mybir.AluOpType.add,
                    )
                    if (name, pos, n, dev) == last_chunk:
                        nc.sync.dma_start(
                            out=params[f"{name}_outT"][m2 * 128 : (m2 + 1) * 128, pos : pos + n],
                            in_=osb[:, m2, :n],
                        )
                if (name, pos, n, dev) != last_chunk:
                    for m2 in range(OUT // 128):
                        nc.sync.dma_start(
                            out=params[f"{name}_outT"][m2 * 128 : (m2 + 1) * 128, pos : pos + n],
                            in_=osb[:, m2, :n],
                        )

    nc.finalize()
    return nc


def _wrap_idx(idx):
    """int array [c] -> wrapped int16 [128, c//16] for dma_gather."""
    c = len(idx)
    w = idx.astype(np.int16).reshape(c // 16, 16).T
    return np.ascontiguousarray(np.tile(w, (8, 1)))


def _pack_w(w):
    """[K, N] -> [128, K//128*N] bf16 (k-tile-major flat columns)"""
    k = w.shape[0]
    return np.ascontiguousarray(
        w.reshape(k // 128, 128, -1).transpose(1, 0, 2).reshape(128, -1).astype(ml_dtypes.bfloat16)
    )


def _prep_b(b):
    """[n] -> [128, n//128]"""
    return np.ascontiguousarray(b.reshape(-1, 128).T)


def kernel(**inputs):
    global LAST_RESULT
    at = np.asarray(inputs["action_type"])
    n_act = at.shape[0]
    out = np.empty((n_act, OUT), dtype=np.float32)

    idx_in = {
        "agv": np.asarray(inputs["agv_idx"]),
        "from": np.asarray(inputs["op_from_idx"]),
        "to": np.asarray(inputs["op_to_idx"]),
        "mach": np.asarray(inputs["machine_idx"]),
    }

    rows = {}
    caps = {}
    pers = {}
    for tcode, (name, tables, K) in zip((2, 3, 1), EXPERTS):
        if tcode == 3:
            r = np.nonzero((at != 0) & (at != 1) & (at != 2))[0]
        else:
            r = np.nonzero(at == tcode)[0]
        rows[name] = r
        pers[name] = -(-max(len(r), 1) // NCORES)  # ceil, >=1
        caps[name] = -(-pers[name] // 128) * 128

    nc = _build(caps)

    # bf16 cast of the embedding tables (shared across cores)
    tab_b = {}
    for tn in ("emb_operation", "emb_machine", "emb_AGV"):
        t = np.asarray(inputs[tn], dtype=np.float32)
        tab_b[f"{tn}_b"] = np.ascontiguousarray(t.astype(ml_dtypes.bfloat16))

    # shared tail of each packed param: [W1 | W2 | biases-as-bf16]
    wtail = {}
    for name, tables, K in EXPERTS:
        b = np.concatenate(
            [_prep_b(np.asarray(inputs[f"{name}_b1"])), _prep_b(np.asarray(inputs[f"{name}_b2"]))],
            axis=1,
        ).astype(np.float32)
        wtail[name] = np.concatenate(
            [
                _pack_w(np.asarray(inputs[f"{name}_W1"])),
                _pack_w(np.asarray(inputs[f"{name}_W2"])),
                np.ascontiguousarray(b).view(ml_dtypes.bfloat16),
            ],
            axis=1,
        )

    in_maps = []
    for core in range(NCORES):
        m = dict(tab_b)
        segs = []
        for name, tables, K in EXPERTS:
            c = caps[name]
            base = min(NA, c)
            r = rows[name]
            per = pers[name]
            shard = r[core * per : (core + 1) * per]
            pad = np.zeros(c, dtype=np.int64)
            pad[: len(shard)] = shard
            # host pre-gather of chunk 0 -> dense feature-major xT
            x0 = np.empty((128, (K // 128) * base), dtype=ml_dtypes.bfloat16)
            for ti, t in enumerate(tables):
                g = tab_b[f"{TABLE_OF[t]}_b"][idx_in[t][pad[:base]]]  # [base, D] bf16
                gt = g.T.reshape(D // 128, 128, base)  # [2, 128, base]
                x0[:, (2 * ti) * base : (2 * ti + 1) * base] = gt[0]
                x0[:, (2 * ti + 1) * base : (2 * ti + 2) * base] = gt[1]
            m[f"{name}_pk"] = np.ascontiguousarray(np.concatenate([x0, wtail[name]], axis=1))
            for t in tables:
                if c > base:
                    segs.append(_wrap_idx(idx_in[t][pad[base:]]))
        if segs:
            m["idx_all"] = np.concatenate(segs, axis=1)
        in_maps.append(m)

    import os

    tmpdir = os.environ.get("BASS_KERNEL_TMPDIR") or None
    res = run_bass_kernel_spmd(nc, in_maps, list(range(NCORES)), tmpdir=tmpdir)
    LAST_RESULT = res

    # assemble
    wait_rows = np.nonzero(at == 0)[0]
    out[wait_rows] = np.asarray(inputs["wait_emb"])[None, :].astype(np.float32)
    for name, tables, K in EXPERTS:
        r = rows[name]
        if len(r) == 0:
            continue
        per = pers[name]
        full = np.concatenate(
            [res.results[core][f"{name}_outT"].T[:per] for core in range(NCORES)],
            axis=0,
        )
        out[r] = full[: len(r)].astype(np.float32)
    return out


# revision 16
# speedup vs baseline: 1.0249x; 1.0249x over previous
"""ActionEncoder Trainium2 kernel (8 NeuronCores, expert-parallel).

Strategy:
- Host groups the 32768 flat actions by action_type (1=pick, 2=transport,
  3=move; type 0 rows are wait_emb and never touch the device), splits each
  group evenly across the 8 cores, and pads each per-core shard to a
  multiple of 128 (pad rows use table row 0 and are discarded).
- Everything on-device is bf16 (rel-err budget 2e-2; measured ~4e-3).
- Startup: the GPSIMD ucode library reload gates SWDGE gathers for the
  first ~14us of the kernel, so the host pre-gathers each expert's FIRST
  chunk (512 rows) and packs it with that expert's W1|W2|biases into ONE
  wide-row bf16 param per expert ([128, ~5-9K] -> 10-18KB DGE descriptors,
  bandwidth-bound); the MLPs start on those while the GPSIMD farm preps
  dma_gather descriptors for the remaining chunks (<=8 gathers so the 8
  SWDGE completion semaphores never recycle). Dummy warmup matmuls on a
  memset tile pull the PE DVFS ladder (~6us per p-state step) early.
- Two bf16 GEMMs per expert with LeakyReLU(0.01)+bias fused on ScalarE
  (hidden stays bf16). Output is written feature-major [256, C] bf16 and
  un-transposed/scattered on the host; the smallest chunk drains last.
- Weights/tables are replicated per core. One SPMD NEFF for all 8 cores.
"""
import sys

import numpy as np

sys.path.insert(0, "/opt/trn_rl_repo")

import ml_dtypes

import concourse.bass as bass
import concourse.bacc as bacc
import concourse.mybir as mybir
import concourse.tile as tile
from concourse import library_config
from concourse.bass_utils import run_bass_kernel_spmd


def _ensure_axon_hooks():
    """Some images lack antenv.axon_hooks; register the ctypes NTFF hook
    shim so run_bass_kernel_spmd's trace path works instead of crashing."""
    try:
        import antenv.axon_hooks  # noqa: F401
        return
    except ImportError:
        pass
    import types

    try:
        import antenv
        from trn_agent_boot.trn_boot import _ntff_profile_via_ctypes

        hook = _ntff_profile_via_ctypes("/opt/axon/libaxon_pjrt.so")
    except Exception:
        return
    mod = types.ModuleType("antenv.axon_hooks")
    state = {"hook": hook}
    mod.get_axon_ntff_profile_hook = lambda: state["hook"]
    mod.set_axon_ntff_profile_hook = lambda h: state.update(hook=h)
    sys.modules["antenv.axon_hooks"] = mod
    antenv.axon_hooks = mod


_ensure_axon_hooks()

D = 256
HID = 512
OUT = 256
NTAB = 8192
NCORES = 8
NA = 512  # max actions per compute chunk (matmul moving dim)
FP32 = mybir.dt.float32
BF16 = mybir.dt.bfloat16
INT16 = mybir.dt.int16

LAST_RESULT = None  # BassKernelResults of the most recent kernel() call

# (name, gathered tables, layer-1 K); trans/move first so the first
# pre-gathered chunks are the cheap ones and pick's device gathers have
# the longest runway.
EXPERTS = (
    ("trans", ("agv", "mach"), 2 * D),
    ("move", ("agv", "mach"), 2 * D),
    ("pick", ("agv", "from", "to", "mach"), 4 * D),
)
TABLE_OF = {"agv": "emb_AGV", "from": "emb_operation", "to": "emb_operation", "mach": "emb_machine"}


def _dev_chunks(c):
    """Device-gathered chunks of (pos, n) covering [NA, c): <=NA each,
    multiples of 128. Chunk 0 ([0, min(NA, c))) is host-pre-gathered."""
    rem = c - min(NA, c)
    if rem == 0:
        return []
    nch = -(-rem // NA)
    base = rem // nch // 128 * 128
    sizes = [base] * (nch - 1) + [rem - base * (nch - 1)]
    sizes.sort(reverse=True)  # largest first -> smallest chunk drains last
    out = []
    pos = NA
    for n in sizes:
        out.append((pos, n))
        pos += n
    return out


def _sched(caps):
    """Interleaved (expert, pos, n, is_dev) chunk order, round-robin across
    experts: every expert's chunk 0 first, then device chunks."""
    lists = {}
    for name, _, _ in EXPERTS:
        c = caps[name]
        lists[name] = [(0, min(NA, c), False)] + [(p, n, True) for p, n in _dev_chunks(c)]
    order = []
    while any(lists.values()):
        for name, _, _ in EXPERTS:
            if lists[name]:
                pos, n, dev = lists[name].pop(0)
                order.append((name, pos, n, dev))
    return order


def _build(caps):
    """Emit the per-core BIR. caps = dict expert -> padded capacity."""
    nc = bacc.Bacc(num_swdge_queues=4)

    tabs = {}
    for tn in ("emb_operation", "emb_machine", "emb_AGV"):
        tabs[tn] = nc.declare_dram_parameter(f"{tn}_b", [NTAB, D], BF16, isOutput=False)

    # one packed bf16 param per expert, wide partition rows so the hw DGE
    # is bandwidth-bound, not descriptor-rate-bound:
    #   [x0 (K/128*base) | W1 (K/128*HID) | W2 (HID/128*OUT) | biases (12)]
    # biases are fp32 bitcast into 12 bf16 columns (b1: HID/128, b2: OUT/128)
    params = {}
    nb = HID // 128 + OUT // 128
    pk_off = {}
    for name, tables, K in EXPERTS:
        c = caps[name]
        base = min(NA, c)
        x0sz = (K // 128) * base
        w1sz = (K // 128) * HID
        w2sz = (HID // 128) * OUT
        pk_off[name] = (x0sz, x0sz + w1sz, x0sz + w1sz + w2sz)
        if name == EXPERTS[0][0]:
            params[f"{name}_pka"] = nc.declare_dram_parameter(
                f"{name}_pka", [128, x0sz + w1sz], BF16, isOutput=False
            )
            params[f"{name}_pkb"] = nc.declare_dram_parameter(
                f"{name}_pkb", [128, w2sz + 2 * nb], BF16, isOutput=False
            )
        else:
            params[f"{name}_pk"] = nc.declare_dram_parameter(
                f"{name}_pk", [128, x0sz + w1sz + w2sz + 2 * nb], BF16, isOutput=False
            )
        params[f"{name}_outT"] = nc.declare_dram_parameter(f"{name}_outT", [OUT, c], BF16, isOutput=True)

    sched = _sched(caps)
    dev_sched = [s for s in sched if s[3]]

    # wrapped int16 index segments, only for device-gathered rows [NA, c)
    seg_off = {}
    off = 0
    for name, tables, K in EXPERTS:
        dev_rows = caps[name] - min(NA, caps[name])
        for t in tables:
            seg_off[(name, t)] = off
            off += dev_rows // 16
    if off:
        params["idx_all"] = nc.declare_dram_parameter("idx_all", [128, off], INT16, isOutput=False)

    qrr = [0]  # SWDGE queue round-robin counter

    with tile.TileContext(nc) as tc:
        with (
            tc.tile_pool(name="wp", bufs=1) as wp,
            tc.tile_pool(name="xp", bufs=2) as xp,
            tc.tile_pool(name="ps", bufs=1, space="PSUM") as ps,
        ):
            if dev_sched:
                nc.gpsimd.load_library(library_config.mlp)

            # --- warm tile built on-chip (no DMA), packed expert params in
            # first-use order, idx after the first expert's param ---
            wpe = wp.tile([128, NA], BF16, name="warm_pe")
            nc.vector.memset(wpe[:], 1.0)
            PK = {}
            idx_all = None
            for i, (name, tables, K) in enumerate(EXPERTS):
                cols = pk_off[name][2] + 2 * nb
                PK[name] = wp.tile([128, cols], BF16, name=f"pk_{name}")
                if i == 0:
                    # split first expert: the first matmul only waits x0|W1
                    o2 = pk_off[name][1]
                    nc.sync.dma_start(out=PK[name][:, :o2], in_=params[f"{name}_pka"][:])
                    nc.sync.dma_start(out=PK[name][:, o2:], in_=params[f"{name}_pkb"][:])
                else:
                    nc.sync.dma_start(out=PK[name][:], in_=params[f"{name}_pk"][:])
                if i == 0 and dev_sched:
                    idx_all = wp.tile([128, off], INT16, name="idx_all")
                    nc.sync.dma_start(out=idx_all[:], in_=params["idx_all"][:])

            # --- device gather preps, issued up-front in compute order ---
            gh = {}
            for name, pos, n, _ in dev_sched:
                tables = dict((e[0], e[1]) for e in EXPERTS)[name]
                base = min(NA, caps[name])
                for t in tables:
                    g = wp.tile([128, D // 128, n], BF16, name=f"g_{name}_{t}_{pos}")
                    so = seg_off[(name, t)]
                    nc.gpsimd.dma_gather(
                        g[:],
                        tabs[TABLE_OF[t]][:],
                        idx_all[:, so + (pos - base) // 16 : so + (pos + n - base) // 16],
                        n,
                        n,
                        D,
                        transpose=True,
                        queue_num=qrr[0] % 4,
                    )
                    qrr[0] += 1
                    gh[(name, t, pos)] = g

            # --- PE p-state warmup: dummy matmuls ramp the clock to 2.4GHz
            # while the first expert's weights/activations stream in ---
            for wi in range(8):
                pw = ps.tile([128, NA], FP32, space="PSUM", tag="pw", bufs=1, name="pw")
                nc.tensor.matmul(
                    out=pw[:], lhsT=wpe[:, :128], rhs=wpe[:], start=True, stop=True
                )

            # --- compute, chunk by chunk ---
            last_chunk = sched[-1]
            expert_of = dict((e[0], e) for e in EXPERTS)
            eidx = dict((e[0], i) for i, e in enumerate(EXPERTS))
            for name, pos, n, dev in sched:
                _, tables, K = expert_of[name]
                base = min(NA, caps[name])
                o1, o2, ob = pk_off[name]

                def rhs1(k):
                    if dev:
                        return gh[(name, tables[k // 2], pos)][:, k % 2, :n]
                    return PK[name][:, k * base : k * base + n]

                def bias_ap(j):
                    return PK[name][:, ob + 2 * j : ob + 2 * j + 2].bitcast(FP32)

                # layer 1: H = Prelu(X @ W1 + b1), feature-major
                hT = xp.tile([128, HID // 128, NA], BF16, tag="hT", name=f"hT_{name}")
                for m in range(HID // 128):
                    p1 = ps.tile([128, NA], FP32, space="PSUM", tag="p1", bufs=3, name="p1")
                    for k in range(K // 128):
                        nc.tensor.matmul(
                            out=p1[:, :n],
                            lhsT=PK[name][:, o1 + k * HID + m * 128 : o1 + k * HID + (m + 1) * 128],
                            rhs=rhs1(k),
                            start=(k == 0),
                            stop=(k == K // 128 - 1),
                        )
                    nc.scalar.activation(
                        out=hT[:, m, :n],
                        in_=p1[:, :n],
                        func=mybir.ActivationFunctionType.Prelu,
                        bias=bias_ap(m),
                        scale=1.0,
                        alpha=0.01,
                    )

                # layer 2: O = H @ W2 + b2, feature-major
                osb = xp.tile([128, OUT // 128, NA], BF16, tag="o", name=f"o_{name}")
                for m2 in range(OUT // 128):
                    p2 = ps.tile([128, NA], FP32, space="PSUM", tag="p2", bufs=3, name="p2")
                    for k2 in range(HID // 128):
                        nc.tensor.matmul(
                            out=p2[:, :n],
                            lhsT=PK[name][:, o2 + k2 * OUT + m2 * 128 : o2 + k2 * OUT + (m2 + 1) * 128],
                            rhs=hT[:, k2, :n],
                            start=(k2 == 0),
                            stop=(k2 == HID // 128 - 1),
                        )
                    nc.vector.tensor_tensor(
                        out=osb[:, m2, :n],
                        in0=p2[:, :n],
                        in1=bias_ap(HID // 128 + m2).to_broadcast([128, n]),
                        op=mybir.AluOpType.add,
                    )
                    if (name, pos, n, dev) == last_chunk:
                        nc.sync.dma_start(
                            out=params[f"{name}_outT"][m2 * 128 : (m2 + 1) * 128, pos : pos + n],
                            in_=osb[:, m2, :n],
                        )
                if (name, pos, n, dev) != last_chunk:
                    for m2 in range(OUT // 128):
                        nc.sync.dma_start(
                            out=params[f"{name}_outT"][m2 * 128 : (m2 + 1) * 128, pos : pos + n],
                            in_=osb[:, m2, :n],
                        )

    nc.finalize()
    return nc


def _wrap_idx(idx):
    """int array [c] -> wrapped int16 [128, c//16] for dma_gather."""
    c = len(idx)
    w = idx.astype(np.int16).reshape(c // 16, 16).T
    return np.ascontiguousarray(np.tile(w, (8, 1)))


def _pack_w(w):
    """[K, N] -> [128, K//128*N] bf16 (k-tile-major flat columns)"""
    k = w.shape[0]
    return np.ascontiguousarray(
        w.reshape(k // 128, 128, -1).transpose(1, 0, 2).reshape(128, -1).astype(ml_dtypes.bfloat16)
    )


def _prep_b(b):
    """[n] -> [128, n//128]"""
    return np.ascontiguousarray(b.reshape(-1, 128).T)


def kernel(**inputs):
    global LAST_RESULT
    at = np.asarray(inputs["action_type"])
    n_act = at.shape[0]
    out = np.empty((n_act, OUT), dtype=np.float32)

    idx_in = {
        "agv": np.asarray(inputs["agv_idx"]),
        "from": np.asarray(inputs["op_from_idx"]),
        "to": np.asarray(inputs["op_to_idx"]),
        "mach": np.asarray(inputs["machine_idx"]),
    }

    rows = {}
    caps = {}
    pers = {}
    for tcode, (name, tables, K) in zip((2, 3, 1), EXPERTS):
        if tcode == 3:
            r = np.nonzero((at != 0) & (at != 1) & (at != 2))[0]
        else:
            r = np.nonzero(at == tcode)[0]
        rows[name] = r
        pers[name] = -(-max(len(r), 1) // NCORES)  # ceil, >=1
        caps[name] = -(-pers[name] // 128) * 128

    nc = _build(caps)

    # bf16 cast of the embedding tables (shared across cores)
    tab_b = {}
    for tn in ("emb_operation", "emb_machine", "emb_AGV"):
        t = np.asarray(inputs[tn], dtype=np.float32)
        tab_b[f"{tn}_b"] = np.ascontiguousarray(t.astype(ml_dtypes.bfloat16))

    # shared tail of each packed param: [W1 | W2 | biases-as-bf16]
    wtail = {}
    for name, tables, K in EXPERTS:
        b = np.concatenate(
            [_prep_b(np.asarray(inputs[f"{name}_b1"])), _prep_b(np.asarray(inputs[f"{name}_b2"]))],
            axis=1,
        ).astype(np.float32)
        wtail[name] = np.concatenate(
            [
                _pack_w(np.asarray(inputs[f"{name}_W1"])),
                _pack_w(np.asarray(inputs[f"{name}_W2"])),
                np.ascontiguousarray(b).view(ml_dtypes.bfloat16),
            ],
            axis=1,
        )

    in_maps = []
    for core in range(NCORES):
        m = dict(tab_b)
        segs = []
        for name, tables, K in EXPERTS:
            c = caps[name]
            base = min(NA, c)
            r = rows[name]
            per = pers[name]
            shard = r[core * per : (core + 1) * per]
            pad = np.zeros(c, dtype=np.int64)
            pad[: len(shard)] = shard
            # host pre-gather of chunk 0 -> dense feature-major xT
            x0 = np.empty((128, (K // 128) * base), dtype=ml_dtypes.bfloat16)
            for ti, t in enumerate(tables):
                g = tab_b[f"{TABLE_OF[t]}_b"][idx_in[t][pad[:base]]]  # [base, D] bf16
                gt = g.T.reshape(D // 128, 128, base)  # [2, 128, base]
                x0[:, (2 * ti) * base : (2 * ti + 1) * base] = gt[0]
                x0[:, (2 * ti + 1) * base : (2 * ti + 2) * base] = gt[1]
            if name == EXPERTS[0][0]:
                w1cols = (K // 128) * HID
                m[f"{name}_pka"] = np.ascontiguousarray(
                    np.concatenate([x0, wtail[name][:, :w1cols]], axis=1)
                )
                m[f"{name}_pkb"] = np.ascontiguousarray(wtail[name][:, w1cols:])
            else:
                m[f"{name}_pk"] = np.ascontiguousarray(np.concatenate([x0, wtail[name]], axis=1))
            for t in tables:
                if c > base:
                    segs.append(_wrap_idx(idx_in[t][pad[base:]]))
        if segs:
            m["idx_all"] = np.concatenate(segs, axis=1)
        in_maps.append(m)

    import os

    tmpdir = os.environ.get("BASS_KERNEL_TMPDIR") or None
    res = run_bass_kernel_spmd(nc, in_maps, list(range(NCORES)), tmpdir=tmpdir)
    LAST_RESULT = res

    # assemble
    wait_rows = np.nonzero(at == 0)[0]
    out[wait_rows] = np.asarray(inputs["wait_emb"])[None, :].astype(np.float32)
    for name, tables, K in EXPERTS:
        r = rows[name]
        if len(r) == 0:
            continue
        per = pers[name]
        full = np.concatenate(
            [res.results[core][f"{name}_outT"].T[:per] for core in range(NCORES)],
            axis=0,
        )
        out[r] = full[: len(r)].astype(np.float32)
    return out
